# revision 11
# baseline (speedup 1.0000x reference)
"""DeepGCN (GENConv softmax-aggr) Trainium2 kernel, 8-core SPMD.

Sharding: by spatial feature position (H*W = 48 -> 6 per core).
Core k owns positions {h*4 + (k%4) : h in [6*(k//4), 6*(k//4)+6)}.
All nodes are local on every core; per-layer BN stats use a tiny (512B)
AllReduce, and the final mean-over-W pooling uses fp16 AllReduces
within each h-half group of 4 cores.

v3: per-dst-chunk processing with
 - ONE indirect DMA per chunk (multi-index offset AP) with CCE-add
   fused (z = h1[src] + he accumulated in the DMA datapath),
 - host-precomputed layer-0 h1 (exact),
 - per-chunk BN-stat accumulation folded into the previous layer,
 - block-diagonal (2-position) GENConv linear with 128-partition
   transposes; feats = h1 + agg folded into the transpose matmuls,
 - engine rebalance: S one-hots + squares on GPSIMD, PSUM drains on ACT,
 - final mean-over-W AllReduce split 4-way and overlapped with layer 3.
"""

import sys
import numpy as np

for p in ("/opt/trn_rl_repo",):
    if p not in sys.path:
        sys.path.insert(0, p)

# ---- problem constants (hardcoded per spec) ----
N_NODES = 5000
N_EDGES = 50000
C = 64
H, W = 12, 4
HW = H * W          # 48
PS = 6              # positions per core
F = C * PS          # 384 features per core-slice  (layout f = p*64 + c)
O = 12
L = 3
NCORES = 8
NCHUNK = 40         # dst-node chunks of 128
NODES_PAD = NCHUNK * 128   # 5120
BN_EPS = 1e-5
EXP_BIAS = -4.0     # e' = exp(m + EXP_BIAS); cancels in softmax, keeps fp16 safe
BN_COUNT = float(N_NODES * HW)

_cache = {}
NL = L          # layers to build (bench knob)
FINAL = True    # include final phase (bench knob)

# knobs
FCOLL_SPLIT = 8        # final AllReduce split (must divide NCHUNK)


def _pos_sets():
    out = []
    for k in range(NCORES):
        w = k % 4
        h0 = 6 * (k // 4)
        out.append(np.array([h * 4 + w for h in range(h0, h0 + 6)]))
    return out


def _host_prep(node_feats, edge_feats, src, dst,
               bn_gamma, bn_beta, gen_W, gen_b, out_W, out_b):
    """Sort/pad edges by dst chunk, build per-core input maps."""
    src = np.asarray(src).astype(np.int64)
    dst = np.asarray(dst).astype(np.int64)
    nf = np.asarray(node_feats, dtype=np.float32).reshape(N_NODES, C, HW)
    ef = np.asarray(edge_feats, dtype=np.float32).reshape(N_EDGES, C, HW)

    order = np.argsort(dst, kind="stable")
    chunk_of = dst[order] // 128

    blk_edges = []          # original-edge indices, -1 for pads
    nblk_per_chunk = []
    for c in range(NCHUNK):
        sel = order[chunk_of == c]
        nb = max(1, (len(sel) + 127) // 128)
        pad = nb * 128 - len(sel)
        blk_edges.append(np.concatenate([sel, np.full(pad, -1, np.int64)]))
        nblk_per_chunk.append(nb)

    eidx = np.concatenate(blk_edges)          # [NBLK*128]
    valid = eidx >= 0
    e_src = np.where(valid, src[np.maximum(eidx, 0)], 0).astype(np.int32)
    chunk_id = np.concatenate(
        [np.full(nblk_per_chunk[c] * 128, c, np.int64) for c in range(NCHUNK)]
    )
    e_ldst = np.where(
        valid, dst[np.maximum(eidx, 0)] - chunk_id * 128, -1
    ).astype(np.float32)

    NBLK = sum(nblk_per_chunk)
    gidx = np.ascontiguousarray(e_src.reshape(NBLK, 128).T)      # [128, NBLK] i32
    ldst = np.ascontiguousarray(e_ldst.reshape(NBLK, 128).T)     # [128, NBLK] f32

    # layer-0 BN affine, exact, from host stats
    g0 = np.asarray(bn_gamma, np.float32)[0]
    be0 = np.asarray(bn_beta, np.float32)[0]
    mu0 = nf.mean(axis=(0, 2))
    var0 = nf.var(axis=(0, 2))
    a0 = g0 / np.sqrt(var0 + BN_EPS)
    b0 = be0 - mu0 * a0
    # exact layer-0 h1 = relu(a0*hv + b0), computed on host
    h1_0 = np.maximum(nf * a0[None, :, None] + b0[None, :, None], 0.0)

    # replicated params
    bng = np.ascontiguousarray(np.asarray(bn_gamma, np.float32).reshape(1, L * C))
    bnb = np.ascontiguousarray(np.asarray(bn_beta, np.float32).reshape(1, L * C))
    # block-diagonal 2-position GEN linear [128, L*128] fp16:
    # row (pp*64+c), col (pp*64+o) = gen_W[l][c, o]
    gw = np.asarray(gen_W, np.float32)
    genw2 = np.zeros((128, L * 128), np.float16)
    for l in range(L):
        genw2[:C, l * 128 : l * 128 + C] = gw[l].astype(np.float16)
        genw2[C:, l * 128 + C : (l + 1) * 128] = gw[l].astype(np.float16)
    genb2 = np.zeros((128, L), np.float32)
    gb = np.asarray(gen_b, np.float32)
    for l in range(L):
        genb2[:C, l] = gb[l]
        genb2[C:, l] = gb[l]
    # final linear, block-diagonal, 0.25 mean-over-W factor folded in
    ow = np.asarray(out_W, np.float32) * 0.25
    outw2 = np.zeros((128, 2 * O), np.float16)
    outw2[:C, :O] = ow.astype(np.float16)
    outw2[C:, O:] = ow.astype(np.float16)
    outb2 = np.zeros((2 * O, 1), np.float32)
    outb2[:O, 0] = np.asarray(out_b, np.float32)
    outb2[O:, 0] = np.asarray(out_b, np.float32)

    ev = eidx[valid]
    pos_sets = _pos_sets()
    in_maps = []
    for k in range(NCORES):
        P_k = pos_sets[k]
        hv0 = np.zeros((NODES_PAD, F), np.float16)
        hv0[:N_NODES] = (
            nf[:, :, P_k].transpose(0, 2, 1).reshape(N_NODES, F).astype(np.float16)
        )
        h1t0 = np.zeros((NODES_PAD, F), np.float16)
        h1t0[:N_NODES] = (
            h1_0[:, :, P_k].transpose(0, 2, 1).reshape(N_NODES, F).astype(np.float16)
        )
        he_full = np.zeros((NBLK * 128, F), np.float16)
        he_full[valid] = (
            ef[ev][:, :, P_k].transpose(0, 2, 1).reshape(len(ev), F).astype(np.float16)
        )
        he_sb = np.ascontiguousarray(
            he_full.reshape(NBLK, 128, F).transpose(1, 0, 2).reshape(128, NBLK * F)
        )
        in_maps.append(
            {
                "hv0": hv0,
                "h1t0": h1t0,
                "he_sb": he_sb,
                "gidx": gidx,
                "ldst": ldst,
                "bn_gamma": bng,
                "bn_beta": bnb,
                "genw2": genw2,
                "genb2": genb2,
                "outw2": outw2,
                "outb2": outb2,
            }
        )
    return in_maps, nblk_per_chunk, pos_sets


def _build_program(nblk_per_chunk):
    import concourse.bacc as bacc
    import concourse.tile as tile
    from concourse import bass, mybir

    dt = mybir.dt
    AF = mybir.ActivationFunctionType
    ALU = mybir.AluOpType
    NBLK = sum(nblk_per_chunk)
    NBMAX = max(nblk_per_chunk)
    blk_base = np.concatenate([[0], np.cumsum(nblk_per_chunk)])
    QROWS = NODES_PAD // FCOLL_SPLIT
    QCHUNK = NCHUNK // FCOLL_SPLIT

    nc = bacc.Bacc(
        "TRN2",
        target_bir_lowering=False,
        debug=False,
        enable_asserts=False,
        num_devices=NCORES,
    )

    hv0_d = nc.dram_tensor("hv0", [NODES_PAD, F], dt.float16, kind="ExternalInput").ap()
    h1t0_d = nc.dram_tensor("h1t0", [NODES_PAD, F], dt.float16, kind="ExternalInput").ap()
    he_d = nc.dram_tensor("he_sb", [128, NBLK * F], dt.float16, kind="ExternalInput").ap()
    gidx_d = nc.dram_tensor("gidx", [128, NBLK], dt.int32, kind="ExternalInput").ap()
    ldst_d = nc.dram_tensor("ldst", [128, NBLK], dt.float32, kind="ExternalInput").ap()
    bng_d = nc.dram_tensor("bn_gamma", [1, L * C], dt.float32, kind="ExternalInput").ap()
    bnb_d = nc.dram_tensor("bn_beta", [1, L * C], dt.float32, kind="ExternalInput").ap()
    genw2_d = nc.dram_tensor("genw2", [128, L * 128], dt.float16, kind="ExternalInput").ap()
    genb2_d = nc.dram_tensor("genb2", [128, L], dt.float32, kind="ExternalInput").ap()
    outw2_d = nc.dram_tensor("outw2", [128, 2 * O], dt.float16, kind="ExternalInput").ap()
    outb2_d = nc.dram_tensor("outb2", [2 * O, 1], dt.float32, kind="ExternalInput").ap()
    out_d = nc.dram_tensor(
        "out_sh", [NODES_PAD, PS * O], dt.float32, kind="ExternalOutput"
    ).ap()

    with tile.TileContext(nc) as tc:
        with (
            tc.tile_pool(name="dram", bufs=1, space="DRAM") as dramp,
            tc.tile_pool(name="resident", bufs=1) as res,
            tc.tile_pool(name="edge", bufs=2) as edgep,
            tc.tile_pool(name="wk", bufs=2) as wk,
            tc.tile_pool(name="small", bufs=2) as small,
            tc.tile_pool(name="psAcc", bufs=1, space="PSUM") as psA,
            tc.tile_pool(name="psLin", bufs=1, space="PSUM") as psL,
            tc.tile_pool(name="psStat", bufs=1, space="PSUM") as psS,
        ):
            h1ts = [h1t0_d]
            for i in range(1, L):
                h1ts.append(
                    dramp.tile([NODES_PAD, F], dt.float16, name=f"h1t{i}")[:]
                )
            bn_in = dramp.tile([1, 128], dt.float32, name="bn_in")
            bn_outs = [
                dramp.tile(
                    [1, 128], dt.float32, addr_space="Shared",
                    name=f"bn_out{l}", tag=f"bn_out{l}",
                )
                for l in range(L)
            ]
            mw_ins = [
                dramp.tile([QROWS, F], dt.float16, name=f"mw_in{q}")
                for q in range(FCOLL_SPLIT)
            ]
            mw_outs = [
                dramp.tile([QROWS, F], dt.float16, name=f"mw_out{q}")
                for q in range(FCOLL_SPLIT)
            ]

            hv = res.tile([128, NCHUNK * F], dt.float16, name="hv")
            h1f = res.tile([128, NCHUNK * F], dt.float16, name="h1f")
            gidx_sb = res.tile([128, NBLK], dt.int32, name="gidx_sb")
            ldst_sb = res.tile([128, NBLK], dt.float32, name="ldst_sb")
            iota_h = res.tile([128, 128], dt.float16, name="iota_h")
            ident_h = res.tile([128, 128], dt.float16, name="ident_h")
            ones_col = res.tile([128, 1], dt.float16, name="ones_col")
            ones_row = res.tile([1, 128], dt.float16, name="ones_row")
            genw2_sb = res.tile([128, L * 128], dt.float16, name="genw2_sb")
            genb2_sb = res.tile([128, L], dt.float32, name="genb2_sb")
            outw2_sb = res.tile([128, 2 * O], dt.float16, name="outw2_sb")
            outb2_sb = res.tile([2 * O, 1], dt.float32, name="outb2_sb")
            bngam = res.tile([1, L * C], dt.float32, name="bngam")
            bnbet = res.tile([1, L * C], dt.float32, name="bnbet")
            a_bc = res.tile([128, F], dt.float16, name="a_bc")
            b_bc = res.tile([128, F], dt.float16, name="b_bc")
            eps_bn = res.tile([1, 1], dt.float32, name="eps_bn")
            pad_mask = res.tile([128, 1], dt.float32, name="pad_mask")
            ebias = res.tile([128, 1], dt.float32, name="ebias")
            s_floor = res.tile([128, 1], dt.float32, name="s_floor")

            iota_i = small.tile([128, 128], dt.int32, name="iota_i", tag="ioi")
            nc.gpsimd.iota(iota_i[:], pattern=[[1, 128]], base=0, channel_multiplier=0)
            nc.vector.tensor_copy(iota_h[:], iota_i[:])
            iota_c = small.tile([128, 1], dt.int32, name="iota_c", tag="ioc")
            nc.gpsimd.iota(iota_c[:], pattern=[[1, 1]], base=0, channel_multiplier=1)
            iota_cf = small.tile([128, 1], dt.float32, name="iota_cf", tag="iocf")
            nc.vector.tensor_copy(iota_cf[:], iota_c[:])
            nc.vector.tensor_scalar(
                out=ident_h[:], in0=iota_h[:], scalar1=iota_cf[:], scalar2=None,
                op0=ALU.is_equal,
            )
            nc.gpsimd.memset(ones_col[:], 1.0)
            nc.gpsimd.memset(ones_row[:], 1.0)
            nc.gpsimd.memset(eps_bn[:], BN_EPS)
            nc.gpsimd.memset(ebias[:], EXP_BIAS)
            nc.gpsimd.memset(s_floor[:], 1e-30)
            nc.vector.tensor_scalar(
                out=pad_mask[:], in0=iota_c[:], scalar1=8, scalar2=None,
                op0=ALU.is_lt,
            )

            nc.sync.dma_start(gidx_sb[:], gidx_d)
            nc.sync.dma_start(ldst_sb[:], ldst_d)
            nc.sync.dma_start(genw2_sb[:], genw2_d)
            nc.sync.dma_start(genb2_sb[:], genb2_d)
            nc.sync.dma_start(outw2_sb[:], outw2_d)
            nc.sync.dma_start(outb2_sb[:], outb2_d)
            nc.sync.dma_start(bngam[:], bng_d)
            nc.sync.dma_start(bnbet[:], bnb_d)
            nc.sync.dma_start(
                hv[:].rearrange("p (k f) -> p k f", f=F),
                hv0_d.rearrange("(k p) f -> p k f", p=128),
            )
            nc.sync.dma_start(
                h1f[:].rearrange("p (k f) -> p k f", f=F),
                h1t0_d.rearrange("(k p) f -> p k f", p=128),
            )

            for l in range(NL):
                need_stats = (l + 1 < NL) or (l + 1 < L and l + 1 < NL + 1 and False)
                need_stats = l + 1 < min(NL, L)

                # ===== barrier: finalize BN stats of hv_l, apply affine =====
                if l > 0:
                    s_c = small.tile([1, C], dt.float32, name="s_c", tag="st")
                    q_c = small.tile([1, C], dt.float32, name="q_c", tag="st2")
                    nc.vector.reduce_sum(
                        s_c[:], ps_sum[:].rearrange("one (p c) -> one c p", c=C),
                        axis=mybir.AxisListType.X,
                    )
                    nc.vector.reduce_sum(
                        q_c[:], ps_sq[:].rearrange("one (p c) -> one c p", c=C),
                        axis=mybir.AxisListType.X,
                    )
                    bn_pack = small.tile([1, 128], dt.float32, name="bn_pack", tag="bnp")
                    nc.vector.tensor_copy(bn_pack[:, 0:C], s_c[:])
                    nc.vector.tensor_copy(bn_pack[:, C : 2 * C], q_c[:])
                    nc.sync.dma_start(bn_in[:], bn_pack[:])
                    nc.gpsimd.collective_compute(
                        "AllReduce",
                        ALU.add,
                        replica_groups=[list(range(NCORES))],
                        ins=[bn_in.opt()],
                        outs=[bn_outs[l].opt()],
                    )
                    bn_g = small.tile([1, 128], dt.float32, name="bn_g", tag="bng")
                    nc.sync.dma_start(bn_g[:], bn_outs[l][:])
                    mu = small.tile([1, C], dt.float32, name="mu", tag="mu")
                    ex2 = small.tile([1, C], dt.float32, name="ex2", tag="ex2")
                    nc.vector.tensor_scalar_mul(mu[:], bn_g[:, 0:C], 1.0 / BN_COUNT)
                    nc.vector.tensor_scalar_mul(ex2[:], bn_g[:, C : 2 * C], 1.0 / BN_COUNT)
                    var = small.tile([1, C], dt.float32, name="var", tag="var")
                    nc.vector.tensor_mul(var[:], mu[:], mu[:])
                    nc.vector.tensor_sub(var[:], ex2[:], var[:])
                    # rstd = exp(-0.5*ln(var+eps)); Log+Exp share an ACT table set
                    lnv = small.tile([1, C], dt.float32, name="lnv", tag="lnv")
                    nc.scalar.activation(lnv[:], var[:], AF.Ln, bias=eps_bn[:])
                    rstd = small.tile([1, C], dt.float32, name="rstd", tag="rstd")
                    nc.scalar.activation(rstd[:], lnv[:], AF.Exp, scale=-0.5)
                    a_c = small.tile([1, C], dt.float32, name="a_cc", tag="ac")
                    b_c = small.tile([1, C], dt.float32, name="b_cc", tag="bc")
                    nc.vector.tensor_mul(a_c[:], bngam[:, l * C : (l + 1) * C], rstd[:])
                    nc.vector.tensor_mul(b_c[:], mu[:], a_c[:])
                    nc.vector.tensor_sub(b_c[:], bnbet[:, l * C : (l + 1) * C], b_c[:])

                    a_row = small.tile([1, F], dt.float16, name="a_row", tag="arow")
                    b_row = small.tile([1, F], dt.float16, name="b_row", tag="brow")
                    for p in range(PS):
                        nc.vector.tensor_copy(a_row[:, p * C : (p + 1) * C], a_c[:])
                        nc.vector.tensor_copy(b_row[:, p * C : (p + 1) * C], b_c[:])
                    ps_ab = psL.tile([128, F], dt.float32, name="ps_ab", tag="t")
                    nc.tensor.matmul(ps_ab[:], ones_row[:], a_row[:], start=True, stop=True)
                    nc.scalar.activation(a_bc[:], ps_ab[:], AF.Identity)
                    ps_ab2 = psL.tile([128, F], dt.float32, name="ps_ab2", tag="o")
                    nc.tensor.matmul(ps_ab2[:], ones_row[:], b_row[:], start=True, stop=True)
                    nc.scalar.activation(b_bc[:], ps_ab2[:], AF.Identity)

                    # phase B sweep: h1 = relu(a*hv + b), DVE/Pool split
                    h1t = h1ts[l]
                    for c in range(NCHUNK):
                        hv_c = hv[:, c * F : (c + 1) * F]
                        h1_c = h1f[:, c * F : (c + 1) * F]
                        z1 = wk.tile([128, F], dt.float16, name="z1", tag="zb")
                        nc.vector.tensor_mul(z1[:], hv_c, a_bc[:])
                        nc.vector.tensor_add(z1[:], z1[:], b_bc[:])
                        nc.vector.tensor_scalar_max(h1_c, z1[:], 0.0)
                        nc.sync.dma_start(h1t[c * 128 : (c + 1) * 128, :], h1_c)

                if need_stats:
                    ps_sum = psS.tile([1, F], dt.float32, name="ps_sum", tag="sum")
                    ps_sq = psS.tile([1, F], dt.float32, name="ps_sq", tag="sq")

                # ===== Phase C: per-chunk edge aggregation + linear + residual =====
                # Software-pipelined: iteration i emits the front stage of
                # chunk i (gather/relu/exp/mul/S/scatter-s) and the tail stage
                # of chunk i-1 (scatter-n/softmax-div/linear/residual), so no
                # engine's program order embeds a cross-engine round trip.
                h1src = h1ts[l]
                front = {}
                s_mats = {}

                def emit_smat(c):
                    b0, b1 = int(blk_base[c]), int(blk_base[c + 1])
                    nb = b1 - b0
                    S_all = edgep.tile(
                        [128, NBMAX * 128], dt.float16, name="S_all", tag="S", bufs=3
                    )
                    for b in range(nb):
                        nc.vector.tensor_scalar(
                            out=S_all[:, b * 128 : (b + 1) * 128],
                            in0=iota_h[:],
                            scalar1=ldst_sb[:, b0 + b : b0 + b + 1],
                            scalar2=None,
                            op0=ALU.is_equal,
                        )
                    s_mats[c] = S_all

                def emit_front(c):
                    b0, b1 = int(blk_base[c]), int(blk_base[c + 1])
                    nb = b1 - b0
                    het = edgep.tile(
                        [128, NBMAX * F], dt.float16, name="het", tag="he", bufs=4
                    )
                    z = het[:, : nb * F]
                    nc.sync.dma_start(z, he_d[:, b0 * F : b1 * F])
                    # z = h1[src] + he: per-block gather fused with CCE add
                    # (HW indirect DMA applies ONE dynamic offset per
                    # partition per instruction)
                    for b in range(nb):
                        nc.gpsimd.indirect_dma_start(
                            out=het[:, b * F : (b + 1) * F],
                            out_offset=None,
                            in_=h1src,
                            in_offset=bass.IndirectOffsetOnAxis(
                                ap=gidx_sb[:, b0 + b : b0 + b + 1], axis=0
                            ),
                            compute_op=ALU.add,
                        )
                    mt = edgep.tile([128, NBMAX * F], dt.float16, name="mt", tag="m")
                    m = mt[:, : nb * F]
                    nc.vector.tensor_scalar_max(m, z, 0.0)
                    et = edgep.tile([128, NBMAX * F], dt.float16, name="et", tag="e")
                    e = et[:, : nb * F]
                    nc.scalar.activation(e, m, AF.Exp, bias=ebias[:])
                    n = z  # overwrite z in place
                    nc.vector.tensor_mul(n, m, e)
                    S_all = s_mats.pop(c)
                    ps_s = psA.tile([128, F], dt.float32, name="ps_s", tag="s", bufs=2)
                    for b in range(nb):
                        nc.tensor.matmul(
                            ps_s[:], S_all[:, b * 128 : (b + 1) * 128],
                            e[:, b * F : (b + 1) * F],
                            start=(b == 0), stop=(b == nb - 1),
                        )
                    front[c] = (nb, n, S_all, ps_s)

                def emit_tail(c):
                    nb, n, S_all, ps_s = front.pop(c)
                    ps_n = psA.tile([128, F], dt.float32, name="ps_n", tag="n")
                    for b in range(nb):
                        nc.tensor.matmul(
                            ps_n[:], S_all[:, b * 128 : (b + 1) * 128],
                            n[:, b * F : (b + 1) * F],
                            start=(b == 0), stop=(b == nb - 1),
                        )
                    hv_c = hv[:, c * F : (c + 1) * F]
                    h1_c = h1f[:, c * F : (c + 1) * F]
                    sadj = wk.tile([128, F], dt.float32, name="sadj", tag="sadj")
                    nc.scalar.activation(sadj[:], ps_s[:], AF.Identity, bias=s_floor[:])
                    rec = wk.tile([128, F], dt.float32, name="rec", tag="rec")
                    nc.vector.reciprocal(rec[:], sadj[:])
                    aggf = wk.tile([128, F], dt.float16, name="aggf", tag="aggf")
                    nc.vector.tensor_mul(aggf[:], ps_n[:], rec[:])
                    # transpose feats = aggf + h1 via accumulating PE matmuls
                    ps_t = psL.tile([128, F], dt.float32, name="ps_t", tag="t")
                    for g in range(3):
                        sl = slice(g * 128, (g + 1) * 128)
                        nc.tensor.matmul(
                            ps_t[:, sl], aggf[:, sl], ident_h[:],
                            start=(g == 0), stop=False,
                        )
                        nc.tensor.matmul(
                            ps_t[:, sl], h1_c[:, sl], ident_h[:],
                            start=False, stop=(g == 2),
                        )
                    fT = wk.tile([128, F], dt.float16, name="fT", tag="fT")
                    nc.scalar.activation(fT[:], ps_t[:], AF.Identity)
                    ps_o = psL.tile([128, F], dt.float32, name="ps_o", tag="o")
                    nc.tensor.matmul(
                        ps_o[:], genw2_sb[:, l * 128 : (l + 1) * 128], fT[:],
                        start=True, stop=True,
                    )
                    oT = wk.tile([128, F], dt.float16, name="oT", tag="oT")
                    nc.scalar.activation(
                        oT[:], ps_o[:], AF.Identity, bias=genb2_sb[:, l : l + 1]
                    )
                    ps_r = psL.tile([128, F], dt.float32, name="ps_r", tag="r")
                    for g in range(3):
                        sl = slice(g * 128, (g + 1) * 128)
                        nc.tensor.matmul(
                            ps_r[:, sl], oT[:, sl], ident_h[:],
                            start=(g == 0), stop=(g == 2),
                        )
                    nc.vector.tensor_add(hv_c, hv_c, ps_r[:])
                    if c == NCHUNK - 1:
                        nc.vector.tensor_scalar_mul(hv_c, hv_c, pad_mask[:])

                    if need_stats:
                        sq = wk.tile([128, F], dt.float16, name="sq", tag="sq")
                        nc.vector.tensor_mul(sq[:], hv_c, hv_c)
                        nc.tensor.matmul(
                            ps_sum[:], ones_col[:], hv_c,
                            start=(c == 0), stop=(c == NCHUNK - 1),
                        )
                        nc.tensor.matmul(
                            ps_sq[:], ones_col[:], sq[:],
                            start=(c == 0), stop=(c == NCHUNK - 1),
                        )

                    if l == L - 1 and FINAL:
                        q, r = divmod(c, QCHUNK)
                        nc.sync.dma_start(
                            mw_ins[q][r * 128 : (r + 1) * 128, :], hv_c
                        )
                        if r == QCHUNK - 1:
                            nc.gpsimd.collective_compute(
                                "AllReduce",
                                ALU.add,
                                replica_groups=[[0, 1, 2, 3], [4, 5, 6, 7]],
                                ins=[mw_ins[q].opt()],
                                outs=[mw_outs[q].opt()],
                            )

                emit_smat(0)
                for i in range(NCHUNK + 1):
                    if i + 1 < NCHUNK:
                        emit_smat(i + 1)
                    if i < NCHUNK:
                        emit_front(i)
                    if i >= 1:
                        emit_tail(i - 1)

            # ===== Final: h_g = (mean_w hv)*hv (0.25 folded into outw2), linear =====
            if FINAL:
                ffront = {}

                def emit_ffront(c):
                    q, r = divmod(c, QCHUNK)
                    hv_c = hv[:, c * F : (c + 1) * F]
                    mean_h = wk.tile([128, F], dt.float16, name="mean_h", tag="meanh")
                    nc.sync.dma_start(
                        mean_h[:], mw_outs[q][r * 128 : (r + 1) * 128, :]
                    )
                    hg = wk.tile([128, F], dt.float16, name="hg", tag="hg")
                    nc.vector.tensor_mul(hg[:], hv_c, mean_h[:])
                    ps_tf = psA.tile([128, F], dt.float32, name="ps_tf", tag="s", bufs=2)
                    for g in range(3):
                        sl = slice(g * 128, (g + 1) * 128)
                        nc.tensor.matmul(
                            ps_tf[:, sl], hg[:, sl], ident_h[:],
                            start=(g == 0), stop=(g == 2),
                        )
                    ffront[c] = ps_tf

                def emit_ftail(c):
                    ps_tf = ffront.pop(c)
                    fTf = wk.tile([128, F], dt.float16, name="fTf", tag="fT")
                    nc.scalar.activation(fTf[:], ps_tf[:], AF.Identity)
                    ps_of = psL.tile([2 * O, F], dt.float32, name="ps_of", tag="o")
                    nc.tensor.matmul(ps_of[:], outw2_sb[:], fTf[:], start=True, stop=True)
                    oTf = wk.tile([2 * O, F], dt.float16, name="oTf", tag="oT")
                    nc.scalar.activation(
                        oTf[:], ps_of[:], AF.Identity, bias=outb2_sb[:]
                    )
                    outps = psL.tile([128, 3 * 2 * O], dt.float32, name="outps", tag="r")
                    for g in range(3):
                        nc.tensor.matmul(
                            outps[:, g * 2 * O : (g + 1) * 2 * O],
                            oTf[:, g * 128 : (g + 1) * 128],
                            ident_h[: 2 * O, : 2 * O],
                            start=(g == 0), stop=(g == 2),
                        )
                    outsb = wk.tile([128, 3 * 2 * O], dt.float32, name="outsb", tag="outsb")
                    nc.vector.tensor_copy(outsb[:], outps[:])
                    nc.sync.dma_start(out_d[c * 128 : (c + 1) * 128, :], outsb[:])

                for i in range(NCHUNK + 1):
                    if i < NCHUNK:
                        emit_ffront(i)
                    if i >= 1:
                        emit_ftail(i - 1)

    nc.compile()
    return nc


def kernel(
    node_feats, edge_feats, src, dst, bn_gamma, bn_beta, gen_W, gen_b, out_W, out_b
):
    from concourse import bass_utils

    in_maps, nblk_per_chunk, pos_sets = _host_prep(
        node_feats, edge_feats, src, dst,
        bn_gamma, bn_beta, gen_W, gen_b, out_W, out_b,
    )

    key = tuple(nblk_per_chunk)
    if key not in _cache:
        _cache[key] = _build_program(nblk_per_chunk)
    nc = _cache[key]

    res = bass_utils.run_bass_kernel_spmd(nc, in_maps, core_ids=list(range(NCORES)))

    out = np.zeros((N_NODES, O, H, W), np.float32)
    for k in range(NCORES):
        o_k = np.asarray(res.results[k]["out_sh"])[:N_NODES]
        for g in range(3):
            for pp in range(2):
                pos = pos_sets[k][2 * g + pp]
                out[:, :, pos // 4, pos % 4] = o_k[
                    :, g * 2 * O + pp * O : g * 2 * O + (pp + 1) * O
                ]
    return out


# revision 12
# speedup vs baseline: 2.1004x; 2.1004x over previous
"""DeepGCN (GENConv softmax-aggr) Trainium2 kernel, 8-core SPMD.

Sharding: by spatial feature position (H*W = 48 -> 6 per core).
Core k owns positions {h*4 + (k%4) : h in [6*(k//4), 6*(k//4)+6)}.
All nodes are local on every core; per-layer BN stats use a tiny (512B)
AllReduce, and the final mean-over-W pooling uses fp16 AllReduces
within each h-half group of 4 cores.

v3: per-dst-chunk processing with
 - ONE indirect DMA per chunk (multi-index offset AP) with CCE-add
   fused (z = h1[src] + he accumulated in the DMA datapath),
 - host-precomputed layer-0 h1 (exact),
 - per-chunk BN-stat accumulation folded into the previous layer,
 - block-diagonal (2-position) GENConv linear with 128-partition
   transposes; feats = h1 + agg folded into the transpose matmuls,
 - engine rebalance: S one-hots + squares on GPSIMD, PSUM drains on ACT,
 - final mean-over-W AllReduce split 4-way and overlapped with layer 3.
"""

import sys
import numpy as np

for p in ("/opt/trn_rl_repo",):
    if p not in sys.path:
        sys.path.insert(0, p)

# ---- problem constants (hardcoded per spec) ----
N_NODES = 5000
N_EDGES = 50000
C = 64
H, W = 12, 4
HW = H * W          # 48
PS = 6              # positions per core
F = C * PS          # 384 features per core-slice  (layout f = p*64 + c)
O = 12
L = 3
NCORES = 8
NCHUNK = 40         # dst-node chunks of 128
NODES_PAD = NCHUNK * 128   # 5120
BN_EPS = 1e-5
EXP_BIAS = -4.0     # e' = exp(m + EXP_BIAS); cancels in softmax, keeps fp16 safe
BN_COUNT = float(N_NODES * HW)

_cache = {}
NL = L          # layers to build (bench knob)
FINAL = True    # include final phase (bench knob)

# knobs
FCOLL_SPLIT = 8        # final AllReduce split (must divide NCHUNK)


def _pos_sets():
    out = []
    for k in range(NCORES):
        w = k % 4
        h0 = 6 * (k // 4)
        out.append(np.array([h * 4 + w for h in range(h0, h0 + 6)]))
    return out


def _host_prep(node_feats, edge_feats, src, dst,
               bn_gamma, bn_beta, gen_W, gen_b, out_W, out_b):
    """Sort/pad edges by dst chunk, build per-core input maps."""
    src = np.asarray(src).astype(np.int64)
    dst = np.asarray(dst).astype(np.int64)
    nf = np.asarray(node_feats, dtype=np.float32).reshape(N_NODES, C, HW)
    ef = np.asarray(edge_feats, dtype=np.float32).reshape(N_EDGES, C, HW)

    order = np.argsort(dst, kind="stable")
    chunk_of = dst[order] // 128

    blk_edges = []          # original-edge indices, -1 for pads
    nblk_per_chunk = []
    for c in range(NCHUNK):
        sel = order[chunk_of == c]
        nb = max(1, (len(sel) + 127) // 128)
        pad = nb * 128 - len(sel)
        blk_edges.append(np.concatenate([sel, np.full(pad, -1, np.int64)]))
        nblk_per_chunk.append(nb)

    eidx = np.concatenate(blk_edges)          # [NBLK*128]
    valid = eidx >= 0
    e_src = np.where(valid, src[np.maximum(eidx, 0)], 0).astype(np.int32)
    chunk_id = np.concatenate(
        [np.full(nblk_per_chunk[c] * 128, c, np.int64) for c in range(NCHUNK)]
    )
    e_ldst = np.where(
        valid, dst[np.maximum(eidx, 0)] - chunk_id * 128, -1
    ).astype(np.float32)

    NBLK = sum(nblk_per_chunk)
    gidx = np.ascontiguousarray(e_src.reshape(NBLK, 128).T)      # [128, NBLK] i32
    ldst = np.ascontiguousarray(e_ldst.reshape(NBLK, 128).T)     # [128, NBLK] f32

    # layer-0 BN affine, exact, from host stats
    g0 = np.asarray(bn_gamma, np.float32)[0]
    be0 = np.asarray(bn_beta, np.float32)[0]
    mu0 = nf.mean(axis=(0, 2))
    var0 = nf.var(axis=(0, 2))
    a0 = g0 / np.sqrt(var0 + BN_EPS)
    b0 = be0 - mu0 * a0
    # exact layer-0 h1 = relu(a0*hv + b0), computed on host
    h1_0 = np.maximum(nf * a0[None, :, None] + b0[None, :, None], 0.0)

    # replicated params
    bng = np.ascontiguousarray(np.asarray(bn_gamma, np.float32).reshape(1, L * C))
    bnb = np.ascontiguousarray(np.asarray(bn_beta, np.float32).reshape(1, L * C))
    # block-diagonal 2-position GEN linear [128, L*128] fp16:
    # row (pp*64+c), col (pp*64+o) = gen_W[l][c, o]
    gw = np.asarray(gen_W, np.float32)
    genw2 = np.zeros((128, L * 128), np.float16)
    for l in range(L):
        genw2[:C, l * 128 : l * 128 + C] = gw[l].astype(np.float16)
        genw2[C:, l * 128 + C : (l + 1) * 128] = gw[l].astype(np.float16)
    genb2 = np.zeros((128, L), np.float32)
    gb = np.asarray(gen_b, np.float32)
    for l in range(L):
        genb2[:C, l] = gb[l]
        genb2[C:, l] = gb[l]
    # final linear, block-diagonal, 0.25 mean-over-W factor folded in
    ow = np.asarray(out_W, np.float32) * 0.25
    outw2 = np.zeros((128, 2 * O), np.float16)
    outw2[:C, :O] = ow.astype(np.float16)
    outw2[C:, O:] = ow.astype(np.float16)
    outb2 = np.zeros((2 * O, 1), np.float32)
    outb2[:O, 0] = np.asarray(out_b, np.float32)
    outb2[O:, 0] = np.asarray(out_b, np.float32)

    ev = eidx[valid]
    pos_sets = _pos_sets()
    in_maps = []
    for k in range(NCORES):
        P_k = pos_sets[k]
        hv0 = np.zeros((NODES_PAD, F), np.float16)
        hv0[:N_NODES] = (
            nf[:, :, P_k].transpose(0, 2, 1).reshape(N_NODES, F).astype(np.float16)
        )
        h1t0 = np.zeros((NODES_PAD, F), np.float16)
        h1t0[:N_NODES] = (
            h1_0[:, :, P_k].transpose(0, 2, 1).reshape(N_NODES, F).astype(np.float16)
        )
        he_full = np.zeros((NBLK * 128, F), np.float16)
        he_full[valid] = (
            ef[ev][:, :, P_k].transpose(0, 2, 1).reshape(len(ev), F).astype(np.float16)
        )
        he_sb = np.ascontiguousarray(
            he_full.reshape(NBLK, 128, F).transpose(1, 0, 2).reshape(128, NBLK * F)
        )
        in_maps.append(
            {
                "hv0": hv0,
                "h1t0": h1t0,
                "he_sb": he_sb,
                "gidx": gidx,
                "ldst": ldst,
                "bn_gamma": bng,
                "bn_beta": bnb,
                "genw2": genw2,
                "genb2": genb2,
                "outw2": outw2,
                "outb2": outb2,
            }
        )
    return in_maps, nblk_per_chunk, pos_sets


def _build_program(nblk_per_chunk):
    import concourse.bacc as bacc
    import concourse.tile as tile
    from concourse import bass, mybir

    dt = mybir.dt
    AF = mybir.ActivationFunctionType
    ALU = mybir.AluOpType
    NBLK = sum(nblk_per_chunk)
    NBMAX = max(nblk_per_chunk)
    blk_base = np.concatenate([[0], np.cumsum(nblk_per_chunk)])
    QROWS = NODES_PAD // FCOLL_SPLIT
    QCHUNK = NCHUNK // FCOLL_SPLIT

    nc = bacc.Bacc(
        "TRN2",
        target_bir_lowering=False,
        debug=False,
        enable_asserts=False,
        num_devices=NCORES,
    )

    hv0_d = nc.dram_tensor("hv0", [NODES_PAD, F], dt.float16, kind="ExternalInput").ap()
    h1t0_d = nc.dram_tensor("h1t0", [NODES_PAD, F], dt.float16, kind="ExternalInput").ap()
    he_d = nc.dram_tensor("he_sb", [128, NBLK * F], dt.float16, kind="ExternalInput").ap()
    gidx_d = nc.dram_tensor("gidx", [128, NBLK], dt.int32, kind="ExternalInput").ap()
    ldst_d = nc.dram_tensor("ldst", [128, NBLK], dt.float32, kind="ExternalInput").ap()
    bng_d = nc.dram_tensor("bn_gamma", [1, L * C], dt.float32, kind="ExternalInput").ap()
    bnb_d = nc.dram_tensor("bn_beta", [1, L * C], dt.float32, kind="ExternalInput").ap()
    genw2_d = nc.dram_tensor("genw2", [128, L * 128], dt.float16, kind="ExternalInput").ap()
    genb2_d = nc.dram_tensor("genb2", [128, L], dt.float32, kind="ExternalInput").ap()
    outw2_d = nc.dram_tensor("outw2", [128, 2 * O], dt.float16, kind="ExternalInput").ap()
    outb2_d = nc.dram_tensor("outb2", [2 * O, 1], dt.float32, kind="ExternalInput").ap()
    out_d = nc.dram_tensor(
        "out_sh", [NODES_PAD, PS * O], dt.float32, kind="ExternalOutput"
    ).ap()

    with tile.TileContext(nc) as tc:
        with (
            tc.tile_pool(name="dram", bufs=1, space="DRAM") as dramp,
            tc.tile_pool(name="resident", bufs=1) as res,
            tc.tile_pool(name="edge", bufs=2) as edgep,
            tc.tile_pool(name="wk", bufs=2) as wk,
            tc.tile_pool(name="small", bufs=2) as small,
            tc.tile_pool(name="psAcc", bufs=1, space="PSUM") as psA,
            tc.tile_pool(name="psLin", bufs=1, space="PSUM") as psL,
            tc.tile_pool(name="psStat", bufs=1, space="PSUM") as psS,
        ):
            h1ts = [h1t0_d]
            for i in range(1, L):
                h1ts.append(
                    dramp.tile([NODES_PAD, F], dt.float16, name=f"h1t{i}")[:]
                )
            bn_in = dramp.tile([1, 128], dt.float32, name="bn_in")
            bn_outs = [
                dramp.tile(
                    [1, 128], dt.float32, addr_space="Shared",
                    name=f"bn_out{l}", tag=f"bn_out{l}",
                )
                for l in range(L)
            ]
            mw_ins = [
                dramp.tile([QROWS, F], dt.float16, name=f"mw_in{q}")
                for q in range(FCOLL_SPLIT)
            ]
            mw_outs = [
                dramp.tile([QROWS, F], dt.float16, name=f"mw_out{q}")
                for q in range(FCOLL_SPLIT)
            ]

            hv = res.tile([128, NCHUNK * F], dt.float16, name="hv")
            h1f = res.tile([128, NCHUNK * F], dt.float16, name="h1f")
            gidx_sb = res.tile([128, NBLK], dt.int32, name="gidx_sb")
            ldst_sb = res.tile([128, NBLK], dt.float32, name="ldst_sb")
            iota_h = res.tile([128, 128], dt.float16, name="iota_h")
            ident_h = res.tile([128, 128], dt.float16, name="ident_h")
            ones_col = res.tile([128, 1], dt.float16, name="ones_col")
            ones_row = res.tile([1, 128], dt.float16, name="ones_row")
            genw2_sb = res.tile([128, L * 128], dt.float16, name="genw2_sb")
            genb2_sb = res.tile([128, L], dt.float32, name="genb2_sb")
            outw2_sb = res.tile([128, 2 * O], dt.float16, name="outw2_sb")
            outb2_sb = res.tile([2 * O, 1], dt.float32, name="outb2_sb")
            bngam = res.tile([1, L * C], dt.float32, name="bngam")
            bnbet = res.tile([1, L * C], dt.float32, name="bnbet")
            a_bc = res.tile([128, F], dt.float16, name="a_bc")
            b_bc = res.tile([128, F], dt.float16, name="b_bc")
            eps_bn = res.tile([1, 1], dt.float32, name="eps_bn")
            pad_mask = res.tile([128, 1], dt.float32, name="pad_mask")
            ebias = res.tile([128, 1], dt.float32, name="ebias")
            s_floor = res.tile([128, 1], dt.float32, name="s_floor")

            iota_i = small.tile([128, 128], dt.int32, name="iota_i", tag="ioi")
            nc.gpsimd.iota(iota_i[:], pattern=[[1, 128]], base=0, channel_multiplier=0)
            nc.vector.tensor_copy(iota_h[:], iota_i[:])
            iota_c = small.tile([128, 1], dt.int32, name="iota_c", tag="ioc")
            nc.gpsimd.iota(iota_c[:], pattern=[[1, 1]], base=0, channel_multiplier=1)
            iota_cf = small.tile([128, 1], dt.float32, name="iota_cf", tag="iocf")
            nc.vector.tensor_copy(iota_cf[:], iota_c[:])
            nc.vector.tensor_scalar(
                out=ident_h[:], in0=iota_h[:], scalar1=iota_cf[:], scalar2=None,
                op0=ALU.is_equal,
            )
            nc.gpsimd.memset(ones_col[:], 1.0)
            nc.gpsimd.memset(ones_row[:], 1.0)
            nc.gpsimd.memset(eps_bn[:], BN_EPS)
            nc.gpsimd.memset(ebias[:], EXP_BIAS)
            nc.gpsimd.memset(s_floor[:], 1e-30)
            nc.vector.tensor_scalar(
                out=pad_mask[:], in0=iota_c[:], scalar1=8, scalar2=None,
                op0=ALU.is_lt,
            )

            nc.sync.dma_start(gidx_sb[:], gidx_d)
            nc.sync.dma_start(ldst_sb[:], ldst_d)
            nc.sync.dma_start(genw2_sb[:], genw2_d)
            nc.sync.dma_start(genb2_sb[:], genb2_d)
            nc.sync.dma_start(outw2_sb[:], outw2_d)
            nc.sync.dma_start(outb2_sb[:], outb2_d)
            nc.sync.dma_start(bngam[:], bng_d)
            nc.sync.dma_start(bnbet[:], bnb_d)
            nc.sync.dma_start(
                hv[:].rearrange("p (k f) -> p k f", f=F),
                hv0_d.rearrange("(k p) f -> p k f", p=128),
            )
            nc.sync.dma_start(
                h1f[:].rearrange("p (k f) -> p k f", f=F),
                h1t0_d.rearrange("(k p) f -> p k f", p=128),
            )

            for l in range(NL):
                need_stats = (l + 1 < NL) or (l + 1 < L and l + 1 < NL + 1 and False)
                need_stats = l + 1 < min(NL, L)

                # ===== barrier: finalize BN stats of hv_l, apply affine =====
                if l > 0:
                    s_c = small.tile([1, C], dt.float32, name="s_c", tag="st")
                    q_c = small.tile([1, C], dt.float32, name="q_c", tag="st2")
                    nc.vector.reduce_sum(
                        s_c[:], ps_sum[:].rearrange("one (p c) -> one c p", c=C),
                        axis=mybir.AxisListType.X,
                    )
                    nc.vector.reduce_sum(
                        q_c[:], ps_sq[:].rearrange("one (p c) -> one c p", c=C),
                        axis=mybir.AxisListType.X,
                    )
                    bn_pack = small.tile([1, 128], dt.float32, name="bn_pack", tag="bnp")
                    nc.vector.tensor_copy(bn_pack[:, 0:C], s_c[:])
                    nc.vector.tensor_copy(bn_pack[:, C : 2 * C], q_c[:])
                    nc.sync.dma_start(bn_in[:], bn_pack[:])
                    nc.gpsimd.collective_compute(
                        "AllReduce",
                        ALU.add,
                        replica_groups=[list(range(NCORES))],
                        ins=[bn_in.opt()],
                        outs=[bn_outs[l].opt()],
                    )
                    bn_g = small.tile([1, 128], dt.float32, name="bn_g", tag="bng")
                    nc.sync.dma_start(bn_g[:], bn_outs[l][:])
                    mu = small.tile([1, C], dt.float32, name="mu", tag="mu")
                    ex2 = small.tile([1, C], dt.float32, name="ex2", tag="ex2")
                    nc.vector.tensor_scalar_mul(mu[:], bn_g[:, 0:C], 1.0 / BN_COUNT)
                    nc.vector.tensor_scalar_mul(ex2[:], bn_g[:, C : 2 * C], 1.0 / BN_COUNT)
                    var = small.tile([1, C], dt.float32, name="var", tag="var")
                    nc.vector.tensor_mul(var[:], mu[:], mu[:])
                    nc.vector.tensor_sub(var[:], ex2[:], var[:])
                    # rstd = exp(-0.5*ln(var+eps)); Log+Exp share an ACT table set
                    lnv = small.tile([1, C], dt.float32, name="lnv", tag="lnv")
                    nc.scalar.activation(lnv[:], var[:], AF.Ln, bias=eps_bn[:])
                    rstd = small.tile([1, C], dt.float32, name="rstd", tag="rstd")
                    nc.scalar.activation(rstd[:], lnv[:], AF.Exp, scale=-0.5)
                    a_c = small.tile([1, C], dt.float32, name="a_cc", tag="ac")
                    b_c = small.tile([1, C], dt.float32, name="b_cc", tag="bc")
                    nc.vector.tensor_mul(a_c[:], bngam[:, l * C : (l + 1) * C], rstd[:])
                    nc.vector.tensor_mul(b_c[:], mu[:], a_c[:])
                    nc.vector.tensor_sub(b_c[:], bnbet[:, l * C : (l + 1) * C], b_c[:])

                    a_row = small.tile([1, F], dt.float16, name="a_row", tag="arow")
                    b_row = small.tile([1, F], dt.float16, name="b_row", tag="brow")
                    for p in range(PS):
                        nc.vector.tensor_copy(a_row[:, p * C : (p + 1) * C], a_c[:])
                        nc.vector.tensor_copy(b_row[:, p * C : (p + 1) * C], b_c[:])
                    ps_ab = psL.tile([128, F], dt.float32, name="ps_ab", tag="t")
                    nc.tensor.matmul(ps_ab[:], ones_row[:], a_row[:], start=True, stop=True)
                    nc.scalar.activation(a_bc[:], ps_ab[:], AF.Identity)
                    ps_ab2 = psL.tile([128, F], dt.float32, name="ps_ab2", tag="o")
                    nc.tensor.matmul(ps_ab2[:], ones_row[:], b_row[:], start=True, stop=True)
                    nc.scalar.activation(b_bc[:], ps_ab2[:], AF.Identity)

                    # phase B sweep: h1 = relu(a*hv + b), DVE/Pool split
                    h1t = h1ts[l]
                    for c in range(NCHUNK):
                        hv_c = hv[:, c * F : (c + 1) * F]
                        h1_c = h1f[:, c * F : (c + 1) * F]
                        z1 = wk.tile([128, F], dt.float16, name="z1", tag="zb")
                        nc.vector.tensor_mul(z1[:], hv_c, a_bc[:])
                        nc.vector.tensor_add(z1[:], z1[:], b_bc[:])
                        nc.vector.tensor_scalar_max(h1_c, z1[:], 0.0)
                        nc.sync.dma_start(h1t[c * 128 : (c + 1) * 128, :], h1_c)

                if need_stats:
                    ps_sum = psS.tile([1, F], dt.float32, name="ps_sum", tag="sum")
                    ps_sq = psS.tile([1, F], dt.float32, name="ps_sq", tag="sq")

                # ===== Phase C: per-chunk edge aggregation + linear + residual =====
                # Software-pipelined: iteration i emits the front stage of
                # chunk i (gather/relu/exp/mul/S/scatter-s) and the tail stage
                # of chunk i-1 (scatter-n/softmax-div/linear/residual), so no
                # engine's program order embeds a cross-engine round trip.
                h1src = h1ts[l]
                front = {}
                s_mats = {}

                def emit_smat(c):
                    b0, b1 = int(blk_base[c]), int(blk_base[c + 1])
                    nb = b1 - b0
                    S_all = edgep.tile(
                        [128, NBMAX * 128], dt.float16, name="S_all", tag="S", bufs=3
                    )
                    for b in range(nb):
                        nc.vector.tensor_scalar(
                            out=S_all[:, b * 128 : (b + 1) * 128],
                            in0=iota_h[:],
                            scalar1=ldst_sb[:, b0 + b : b0 + b + 1],
                            scalar2=None,
                            op0=ALU.is_equal,
                        )
                    s_mats[c] = S_all

                def emit_front(c):
                    b0, b1 = int(blk_base[c]), int(blk_base[c + 1])
                    nb = b1 - b0
                    het = edgep.tile(
                        [128, NBMAX * F], dt.float16, name="het", tag="he", bufs=2
                    )
                    he = het[:, : nb * F]
                    nc.sync.dma_start(he, he_d[:, b0 * F : b1 * F])
                    # per-block bypass gathers (HW indirect DMA applies ONE
                    # dynamic offset per partition per instruction)
                    zt = edgep.tile(
                        [128, NBMAX * F], dt.float16, name="zt", tag="z", bufs=3
                    )
                    z = zt[:, : nb * F]
                    for b in range(nb):
                        nc.gpsimd.indirect_dma_start(
                            out=zt[:, b * F : (b + 1) * F],
                            out_offset=None,
                            in_=h1src,
                            in_offset=bass.IndirectOffsetOnAxis(
                                ap=gidx_sb[:, b0 + b : b0 + b + 1], axis=0
                            ),
                        )
                    nc.vector.tensor_add(z, z, he)
                    mt = edgep.tile([128, NBMAX * F], dt.float16, name="mt", tag="m")
                    m = mt[:, : nb * F]
                    nc.vector.tensor_scalar_max(m, z, 0.0)
                    et = edgep.tile([128, NBMAX * F], dt.float16, name="et", tag="e")
                    e = et[:, : nb * F]
                    nc.scalar.activation(e, m, AF.Exp, bias=ebias[:])
                    n = z  # overwrite z in place
                    nc.vector.tensor_mul(n, m, e)
                    S_all = s_mats.pop(c)
                    ps_s = psA.tile([128, F], dt.float32, name="ps_s", tag="s", bufs=2)
                    for b in range(nb):
                        nc.tensor.matmul(
                            ps_s[:], S_all[:, b * 128 : (b + 1) * 128],
                            e[:, b * F : (b + 1) * F],
                            start=(b == 0), stop=(b == nb - 1),
                        )
                    front[c] = (nb, n, S_all, ps_s)

                def emit_tail(c):
                    nb, n, S_all, ps_s = front.pop(c)
                    ps_n = psA.tile([128, F], dt.float32, name="ps_n", tag="n")
                    for b in range(nb):
                        nc.tensor.matmul(
                            ps_n[:], S_all[:, b * 128 : (b + 1) * 128],
                            n[:, b * F : (b + 1) * F],
                            start=(b == 0), stop=(b == nb - 1),
                        )
                    hv_c = hv[:, c * F : (c + 1) * F]
                    h1_c = h1f[:, c * F : (c + 1) * F]
                    sadj = wk.tile([128, F], dt.float32, name="sadj", tag="sadj")
                    nc.scalar.activation(sadj[:], ps_s[:], AF.Identity, bias=s_floor[:])
                    rec = wk.tile([128, F], dt.float32, name="rec", tag="rec")
                    nc.vector.reciprocal(rec[:], sadj[:])
                    aggf = wk.tile([128, F], dt.float16, name="aggf", tag="aggf")
                    nc.vector.tensor_mul(aggf[:], ps_n[:], rec[:])
                    # transpose feats = aggf + h1 via accumulating PE matmuls
                    ps_t = psL.tile([128, F], dt.float32, name="ps_t", tag="t")
                    for g in range(3):
                        sl = slice(g * 128, (g + 1) * 128)
                        nc.tensor.matmul(
                            ps_t[:, sl], aggf[:, sl], ident_h[:],
                            start=(g == 0), stop=False,
                        )
                        nc.tensor.matmul(
                            ps_t[:, sl], h1_c[:, sl], ident_h[:],
                            start=False, stop=(g == 2),
                        )
                    fT = wk.tile([128, F], dt.float16, name="fT", tag="fT")
                    nc.scalar.activation(fT[:], ps_t[:], AF.Identity)
                    ps_o = psL.tile([128, F], dt.float32, name="ps_o", tag="o")
                    nc.tensor.matmul(
                        ps_o[:], genw2_sb[:, l * 128 : (l + 1) * 128], fT[:],
                        start=True, stop=True,
                    )
                    oT = wk.tile([128, F], dt.float16, name="oT", tag="oT")
                    nc.scalar.activation(
                        oT[:], ps_o[:], AF.Identity, bias=genb2_sb[:, l : l + 1]
                    )
                    ps_r = psL.tile([128, F], dt.float32, name="ps_r", tag="r")
                    for g in range(3):
                        sl = slice(g * 128, (g + 1) * 128)
                        nc.tensor.matmul(
                            ps_r[:, sl], oT[:, sl], ident_h[:],
                            start=(g == 0), stop=(g == 2),
                        )
                    nc.vector.tensor_add(hv_c, hv_c, ps_r[:])
                    if c == NCHUNK - 1:
                        nc.vector.tensor_scalar_mul(hv_c, hv_c, pad_mask[:])

                    if need_stats:
                        sq = wk.tile([128, F], dt.float16, name="sq", tag="sq")
                        nc.vector.tensor_mul(sq[:], hv_c, hv_c)
                        nc.tensor.matmul(
                            ps_sum[:], ones_col[:], hv_c,
                            start=(c == 0), stop=(c == NCHUNK - 1),
                        )
                        nc.tensor.matmul(
                            ps_sq[:], ones_col[:], sq[:],
                            start=(c == 0), stop=(c == NCHUNK - 1),
                        )

                    if l == L - 1 and FINAL:
                        q, r = divmod(c, QCHUNK)
                        nc.sync.dma_start(
                            mw_ins[q][r * 128 : (r + 1) * 128, :], hv_c
                        )
                        if r == QCHUNK - 1:
                            nc.gpsimd.collective_compute(
                                "AllReduce",
                                ALU.add,
                                replica_groups=[[0, 1, 2, 3], [4, 5, 6, 7]],
                                ins=[mw_ins[q].opt()],
                                outs=[mw_outs[q].opt()],
                            )

                emit_smat(0)
                for i in range(NCHUNK + 1):
                    if i + 1 < NCHUNK:
                        emit_smat(i + 1)
                    if i < NCHUNK:
                        emit_front(i)
                    if i >= 1:
                        emit_tail(i - 1)

            # ===== Final: h_g = (mean_w hv)*hv (0.25 folded into outw2), linear =====
            if FINAL:
                ffront = {}

                def emit_ffront(c):
                    q, r = divmod(c, QCHUNK)
                    hv_c = hv[:, c * F : (c + 1) * F]
                    mean_h = wk.tile([128, F], dt.float16, name="mean_h", tag="meanh")
                    nc.sync.dma_start(
                        mean_h[:], mw_outs[q][r * 128 : (r + 1) * 128, :]
                    )
                    hg = wk.tile([128, F], dt.float16, name="hg", tag="hg")
                    nc.vector.tensor_mul(hg[:], hv_c, mean_h[:])
                    ps_tf = psA.tile([128, F], dt.float32, name="ps_tf", tag="s", bufs=2)
                    for g in range(3):
                        sl = slice(g * 128, (g + 1) * 128)
                        nc.tensor.matmul(
                            ps_tf[:, sl], hg[:, sl], ident_h[:],
                            start=(g == 0), stop=(g == 2),
                        )
                    ffront[c] = ps_tf

                def emit_ftail(c):
                    ps_tf = ffront.pop(c)
                    fTf = wk.tile([128, F], dt.float16, name="fTf", tag="fT")
                    nc.scalar.activation(fTf[:], ps_tf[:], AF.Identity)
                    ps_of = psL.tile([2 * O, F], dt.float32, name="ps_of", tag="o")
                    nc.tensor.matmul(ps_of[:], outw2_sb[:], fTf[:], start=True, stop=True)
                    oTf = wk.tile([2 * O, F], dt.float16, name="oTf", tag="oT")
                    nc.scalar.activation(
                        oTf[:], ps_of[:], AF.Identity, bias=outb2_sb[:]
                    )
                    outps = psL.tile([128, 3 * 2 * O], dt.float32, name="outps", tag="r")
                    for g in range(3):
                        nc.tensor.matmul(
                            outps[:, g * 2 * O : (g + 1) * 2 * O],
                            oTf[:, g * 128 : (g + 1) * 128],
                            ident_h[: 2 * O, : 2 * O],
                            start=(g == 0), stop=(g == 2),
                        )
                    outsb = wk.tile([128, 3 * 2 * O], dt.float32, name="outsb", tag="outsb")
                    nc.vector.tensor_copy(outsb[:], outps[:])
                    nc.sync.dma_start(out_d[c * 128 : (c + 1) * 128, :], outsb[:])

                for i in range(NCHUNK + 1):
                    if i < NCHUNK:
                        emit_ffront(i)
                    if i >= 1:
                        emit_ftail(i - 1)

    nc.compile()
    return nc


def kernel(
    node_feats, edge_feats, src, dst, bn_gamma, bn_beta, gen_W, gen_b, out_W, out_b
):
    from concourse import bass_utils

    in_maps, nblk_per_chunk, pos_sets = _host_prep(
        node_feats, edge_feats, src, dst,
        bn_gamma, bn_beta, gen_W, gen_b, out_W, out_b,
    )

    key = tuple(nblk_per_chunk)
    if key not in _cache:
        _cache[key] = _build_program(nblk_per_chunk)
    nc = _cache[key]

    res = bass_utils.run_bass_kernel_spmd(nc, in_maps, core_ids=list(range(NCORES)))

    out = np.zeros((N_NODES, O, H, W), np.float32)
    for k in range(NCORES):
        o_k = np.asarray(res.results[k]["out_sh"])[:N_NODES]
        for g in range(3):
            for pp in range(2):
                pos = pos_sets[k][2 * g + pp]
                out[:, :, pos // 4, pos % 4] = o_k[
                    :, g * 2 * O + pp * O : g * 2 * O + (pp + 1) * O
                ]
    return out
